# revision 6
# baseline (speedup 1.0000x reference)
"""GAT tree-aggregation on 8 Trainium2 NeuronCores (Bass/Tile kernel).

Sharding: pure data parallel over batch B=1024 -> 128 samples per core
(batch maps exactly onto the 128 SBUF partitions); params replicated.

The axon tunnel moves ~42 MB/s, so wall-clock is dominated by host->device
transfer. x2 (256MB fp32) is shipped as per-row int8 + fp16 scales
(65.5MB + 0.5MB); x1/x0/params ship as fp16. Device math: fp16 products,
fp32 accumulation -> final rel err ~1e-2 (budget 2e-2).

Device kernel layout (per core, b=128 on partitions):
  - attention logits / softmax / weighted sums on DVE+ACT in batch-major
    layout (tensor_tensor mult + tensor_reduce over the strided axis)
  - per-head projections on the PE via identity-transposes:
    z (b, h, f) -T-> (f, b) -matmul w[h]-> (d, b) -T-> (b, d)
  - final fc stays in transposed layout until one last PE transpose.
"""
import os
import sys
import threading

import numpy as np

for _p in ("/opt/trn_rl_repo", "/root/.axon_site/_ro/trn_rl_repo"):
    if os.path.isdir(_p) and _p not in sys.path:
        sys.path.append(_p)

N_CORES = 8
B = 1024
BC = B // N_CORES          # 128 samples per core == SBUF partitions
P1, S2 = 10, 25            # tree fan-outs
F, H, D = 256, 4, 128
F1 = H * D                 # 512
NEG_SLOPE = 0.2


# ---------------------------------------------------------------------------
# Device program
# ---------------------------------------------------------------------------

def build_program():
    """Build the per-core Bass program. Returns the compiled Bacc object."""
    import concourse.bacc as bacc
    import concourse.tile as tile
    from concourse import mybir
    from concourse.masks import make_identity
    from contextlib import ExitStack

    f16 = mybir.dt.float16
    f32 = mybir.dt.float32
    i8 = mybir.dt.int8
    AF = mybir.ActivationFunctionType
    ALU = mybir.AluOpType
    AX = mybir.AxisListType

    nc = bacc.Bacc("TRN2", target_bir_lowering=False, debug=False,
                   num_devices=N_CORES)

    x2q = nc.dram_tensor("x2q", [BC, P1 * S2, F], i8, kind="ExternalInput").ap()
    x2s = nc.dram_tensor("x2s", [BC, P1 * S2], f16, kind="ExternalInput").ap()
    x1 = nc.dram_tensor("x1", [BC, P1, F], f16, kind="ExternalInput").ap()
    x0 = nc.dram_tensor("x0", [BC, F], f16, kind="ExternalInput").ap()
    w0 = nc.dram_tensor("w0", [H, F, D], f16, kind="ExternalInput").ap()
    a0s = nc.dram_tensor("a0s", [H, F], f16, kind="ExternalInput").ap()
    a0n = nc.dram_tensor("a0n", [H, F], f16, kind="ExternalInput").ap()
    w1 = nc.dram_tensor("w1", [H, F1, D], f16, kind="ExternalInput").ap()
    a1s = nc.dram_tensor("a1s", [H, F1], f16, kind="ExternalInput").ap()
    a1n = nc.dram_tensor("a1n", [H, F1], f16, kind="ExternalInput").ap()
    fcw = nc.dram_tensor("fcw", [F1, F], f16, kind="ExternalInput").ap()
    out = nc.dram_tensor("out", [BC, F], f32, kind="ExternalOutput").ap()

    import concourse.bass as bass

    def bcastP(ap, parts=128):
        # replicate a DRAM row block across all 128 partitions (DMA source)
        return bass.AP(tensor=ap.tensor, offset=ap.offset,
                       ap=[[0, parts]] + [list(d) for d in ap.ap])

    with tile.TileContext(nc) as tc:
        with ExitStack() as ctx:
            const = ctx.enter_context(tc.tile_pool(name="const", bufs=1))
            xpool = ctx.enter_context(tc.tile_pool(name="xpool", bufs=1))
            qpool = ctx.enter_context(tc.tile_pool(name="qpool", bufs=2))
            xfpool = ctx.enter_context(tc.tile_pool(name="xfpool", bufs=2))
            tmpbig = ctx.enter_context(tc.tile_pool(name="tmpbig", bufs=2))  # shared (25,256) f16 temps
            tmp512 = ctx.enter_context(tc.tile_pool(name="tmp512", bufs=2))
            pers = ctx.enter_context(tc.tile_pool(name="pers", bufs=1))
            small = ctx.enter_context(tc.tile_pool(name="small", bufs=2))
            zpool = ctx.enter_context(tc.tile_pool(name="zpool", bufs=2))
            zhpool = ctx.enter_context(tc.tile_pool(name="zhpool", bufs=2))
            ztpool = ctx.enter_context(tc.tile_pool(name="ztpool", bufs=10))
            htpool = ctx.enter_context(tc.tile_pool(name="htpool", bufs=6))
            hpool = ctx.enter_context(tc.tile_pool(name="hpool", bufs=1))
            opool = ctx.enter_context(tc.tile_pool(name="opool", bufs=1))
            psT = ctx.enter_context(tc.tile_pool(name="psT", bufs=2, space="PSUM"))
            psM = ctx.enter_context(tc.tile_pool(name="psM", bufs=4, space="PSUM"))

            # ---- constants ----
            ident16 = const.tile([128, 128], f16)
            make_identity(nc, ident16)
            ident32 = const.tile([128, 128], f32)
            make_identity(nc, ident32)

            a0s_b = const.tile([128, H, F], f16)
            nc.sync.dma_start(out=a0s_b, in_=bcastP(a0s))
            a0n_b = const.tile([128, H, F], f16)
            nc.sync.dma_start(out=a0n_b, in_=bcastP(a0n))
            a1s_b = const.tile([128, H, F1], f16)
            nc.sync.dma_start(out=a1s_b, in_=bcastP(a1s))
            a1n_b = const.tile([128, H, F1], f16)
            nc.sync.dma_start(out=a1n_b, in_=bcastP(a1n))

            w0_t = const.tile([128, H, 2, D], f16)      # (f-chunk part, h, fh, d)
            for h in range(H):
                for fh in range(2):
                    nc.sync.dma_start(out=w0_t[:, h, fh, :],
                                      in_=w0[h, fh * 128:(fh + 1) * 128, :])
            w1_t = const.tile([128, H, 4, D], f16)
            for h in range(H):
                for fc in range(4):
                    nc.sync.dma_start(out=w1_t[:, h, fc, :],
                                      in_=w1[h, fc * 128:(fc + 1) * 128, :])
            fcw_t = const.tile([128, 4, 2, 128], f16)   # (f1-chunk part, fc, mh, d2)
            for fc in range(4):
                for mh in range(2):
                    nc.sync.dma_start(
                        out=fcw_t[:, fc, mh, :],
                        in_=fcw[fc * 128:(fc + 1) * 128, mh * 128:(mh + 1) * 128])

            x1_s = xpool.tile([128, P1, F], f16)
            nc.sync.dma_start(out=x1_s, in_=x1)
            x0_s = xpool.tile([128, F], f16)
            nc.sync.dma_start(out=x0_s, in_=x0)
            x2s_s = xpool.tile([128, P1 * S2], f16)
            nc.sync.dma_start(out=x2s_s, in_=x2s)

            h1_s = hpool.tile([128, P1, F1], f16)
            h0_s = hpool.tile([128, F1], f16)

            def dots(dst, h, src, avec_b, n, f):
                """dst[:, h, :n] = sum_f src(128,n,f) * avec_b[:,h,:f] (f16 tmp)."""
                pool = tmpbig if (n, f) == (S2, F) else tmp512
                t = pool.tile([128, n, f], f16, tag=f"t{n}x{f}")
                nc.vector.tensor_tensor(
                    out=t, in0=src,
                    in1=avec_b[:, h, :][:, None, :].broadcast_to([128, n, f]),
                    op=ALU.mult)
                nc.vector.tensor_reduce(out=dst, in_=t, axis=AX.X, op=ALU.add)

            def exp_lrelu(u, shp, tag):
                """u <- exp(leaky_relu(u)) ; sim-safe composition."""
                t = small.tile([128] + shp, f32, tag="lr" + tag)
                nc.scalar.mul(t, u, NEG_SLOPE)
                nc.vector.tensor_tensor(out=u, in0=u, in1=t, op=ALU.max)
                nc.scalar.activation(u, u, AF.Exp)

            # ---- pre-stage: per-node logits of x1 against a0 ----
            ls0 = pers.tile([128, H, P1], f32)    # x1 . a0_self
            lns0 = pers.tile([128, H, P1], f32)   # x1 . a0_neigh
            for h in range(H):
                dots(ls0[:, h, :], h, x1_s, a0s_b, P1, F)
                dots(lns0[:, h, :], h, x1_s, a0n_b, P1, F)
            es0 = pers.tile([128, H, P1], f32)    # exp(lrelu(ls+ln_self)) lvl-1 self
            nc.vector.tensor_tensor(out=es0, in0=ls0, in1=lns0, op=ALU.add)
            exp_lrelu(es0, [H, P1], "hp")

            # =========== level-1 GAT layer-0: h1[b, p] over x2 neighbors =====
            for p in range(P1):
                qt = qpool.tile([128, S2, F], i8)
                nc.sync.dma_start(out=qt, in_=x2q[:, p * S2:(p + 1) * S2, :])
                xf = xfpool.tile([128, S2, F], f16)
                nc.vector.tensor_copy(xf, qt)  # int8 -> f16 cast
                nc.vector.tensor_tensor(
                    out=xf, in0=xf,
                    in1=x2s_s[:, p * S2:(p + 1) * S2][:, :, None]
                        .broadcast_to([128, S2, F]),
                    op=ALU.mult)

                ln = small.tile([128, H, S2], f32)
                for h in range(H):
                    dots(ln[:, h, :], h, xf, a0n_b, S2, F)
                en = small.tile([128, H, S2], f32)
                nc.vector.tensor_tensor(
                    out=en, in0=ln,
                    in1=ls0[:, :, p][:, :, None].broadcast_to([128, H, S2]),
                    op=ALU.add)
                exp_lrelu(en, [H, S2], "hs")

                den = small.tile([128, H], f32)
                nc.vector.tensor_reduce(out=den, in_=en, axis=AX.X, op=ALU.add)
                nc.vector.tensor_tensor(out=den, in0=den, in1=es0[:, :, p],
                                        op=ALU.add)
                rden = small.tile([128, H], f32)
                nc.vector.reciprocal(rden, den)
                enf = small.tile([128, H, S2], f16)
                nc.vector.tensor_tensor(
                    out=enf, in0=en,
                    in1=rden[:, :, None].broadcast_to([128, H, S2]), op=ALU.mult)
                esf = small.tile([128, H], f16)
                nc.vector.tensor_tensor(out=esf, in0=es0[:, :, p], in1=rden,
                                        op=ALU.mult)

                z = zpool.tile([128, H, F], f16)
                nc.vector.tensor_tensor(
                    out=z,
                    in0=x1_s[:, p, :][:, None, :].broadcast_to([128, H, F]),
                    in1=esf[:, :, None].broadcast_to([128, H, F]), op=ALU.mult)
                for h in range(H):
                    wt = tmpbig.tile([128, S2, F], f16, tag=f"t{S2}x{F}")
                    nc.vector.tensor_tensor(
                        out=wt, in0=xf,
                        in1=enf[:, h, :][:, :, None].broadcast_to([128, S2, F]),
                        op=ALU.mult)
                    zh = zhpool.tile([128, F], f32, tag="zh")
                    nc.vector.tensor_reduce(out=zh, in_=wt.transpose([0, 2, 1]),
                                            axis=AX.X, op=ALU.add)
                    nc.vector.tensor_tensor(out=z[:, h, :], in0=z[:, h, :],
                                            in1=zh, op=ALU.add)

                # project: h1[b, p, (h d)] = z[b, h, :] @ w0[h]
                zT = []
                for h in range(H):
                    row = []
                    for fh in range(2):
                        pt = psT.tile([128, 128], f16, tag="psT")
                        nc.tensor.transpose(pt, z[:, h, fh * 128:(fh + 1) * 128],
                                            ident16)
                        zt = ztpool.tile([128, 128], f16, tag="zt")
                        nc.scalar.copy(zt, pt)
                        row.append(zt)
                    zT.append(row)
                for h in range(H):
                    hp = psM.tile([128, 128], f32, tag="psM")
                    nc.tensor.matmul(hp, w0_t[:, h, 0, :], zT[h][0],
                                     start=True, stop=False)
                    nc.tensor.matmul(hp, w0_t[:, h, 1, :], zT[h][1],
                                     start=False, stop=True)
                    ht = htpool.tile([128, 128], f16, tag="ht")
                    nc.scalar.copy(ht, hp)
                    bt = psT.tile([128, 128], f16, tag="psT")
                    nc.tensor.transpose(bt, ht, ident16)
                    nc.scalar.copy(h1_s[:, p, h * 128:(h + 1) * 128], bt)

            # =========== level-0 GAT layer-0: h0[b] over x1 neighbors ========
            lsx0 = pers.tile([128, H], f32)
            lnx0 = pers.tile([128, H], f32)
            for h in range(H):
                dots(lsx0[:, h:h+1], h, x0_s[:, None, :], a0s_b, 1, F)
                dots(lnx0[:, h:h+1], h, x0_s[:, None, :], a0n_b, 1, F)

            e0n = pers.tile([128, H, P1], f32)
            nc.vector.tensor_tensor(
                out=e0n, in0=lns0,
                in1=lsx0[:, :, None].broadcast_to([128, H, P1]), op=ALU.add)
            exp_lrelu(e0n, [H, P1], "hp")
            e0s = pers.tile([128, H], f32)
            nc.vector.tensor_tensor(out=e0s, in0=lsx0, in1=lnx0, op=ALU.add)
            exp_lrelu(e0s, [H], "h")

            den0 = pers.tile([128, H], f32)
            nc.vector.tensor_reduce(out=den0, in_=e0n, axis=AX.X, op=ALU.add)
            nc.vector.tensor_tensor(out=den0, in0=den0, in1=e0s, op=ALU.add)
            r0 = pers.tile([128, H], f32)
            nc.vector.reciprocal(r0, den0)
            e0nf = pers.tile([128, H, P1], f16)
            nc.vector.tensor_tensor(
                out=e0nf, in0=e0n,
                in1=r0[:, :, None].broadcast_to([128, H, P1]), op=ALU.mult)
            e0sf = pers.tile([128, H], f16)
            nc.vector.tensor_tensor(out=e0sf, in0=e0s, in1=r0, op=ALU.mult)

            z0 = zpool.tile([128, H, F], f16, tag="z0")
            nc.vector.tensor_tensor(
                out=z0, in0=x0_s[:, None, :].broadcast_to([128, H, F]),
                in1=e0sf[:, :, None].broadcast_to([128, H, F]), op=ALU.mult)
            for h in range(H):
                wt0 = tmp512.tile([128, P1, F], f16, tag=f"t{P1}x{F}")
                nc.vector.tensor_tensor(
                    out=wt0, in0=x1_s,
                    in1=e0nf[:, h, :][:, :, None].broadcast_to([128, P1, F]),
                    op=ALU.mult)
                zh0 = zhpool.tile([128, F], f32, tag="zh")
                nc.vector.tensor_reduce(out=zh0, in_=wt0.transpose([0, 2, 1]),
                                        axis=AX.X, op=ALU.add)
                nc.vector.tensor_tensor(out=z0[:, h, :], in0=z0[:, h, :],
                                        in1=zh0, op=ALU.add)

            z0T = []
            for h in range(H):
                row = []
                for fh in range(2):
                    pt = psT.tile([128, 128], f16, tag="psT")
                    nc.tensor.transpose(pt, z0[:, h, fh * 128:(fh + 1) * 128],
                                        ident16)
                    zt = ztpool.tile([128, 128], f16, tag="zt")
                    nc.scalar.copy(zt, pt)
                    row.append(zt)
                z0T.append(row)
            for h in range(H):
                hp = psM.tile([128, 128], f32, tag="psM")
                nc.tensor.matmul(hp, w0_t[:, h, 0, :], z0T[h][0],
                                 start=True, stop=False)
                nc.tensor.matmul(hp, w0_t[:, h, 1, :], z0T[h][1],
                                 start=False, stop=True)
                ht = htpool.tile([128, 128], f16, tag="ht")
                nc.scalar.copy(ht, hp)
                bt = psT.tile([128, 128], f16, tag="psT")
                nc.tensor.transpose(bt, ht, ident16)
                nc.scalar.copy(h0_s[:, h * 128:(h + 1) * 128], bt)

            # =========== level-0 GAT layer-1: h0' over h1 neighbors ==========
            ls1 = pers.tile([128, H], f32)
            ln1self = pers.tile([128, H], f32)
            for h in range(H):
                dots(ls1[:, h:h+1], h, h0_s[:, None, :], a1s_b, 1, F1)
                dots(ln1self[:, h:h+1], h, h0_s[:, None, :], a1n_b, 1, F1)
            ln1 = pers.tile([128, H, P1], f32)
            for h in range(H):
                dots(ln1[:, h, :], h, h1_s, a1n_b, P1, F1)

            e1n = pers.tile([128, H, P1], f32)
            nc.vector.tensor_tensor(
                out=e1n, in0=ln1,
                in1=ls1[:, :, None].broadcast_to([128, H, P1]), op=ALU.add)
            exp_lrelu(e1n, [H, P1], "hp")
            e1s = pers.tile([128, H], f32)
            nc.vector.tensor_tensor(out=e1s, in0=ls1, in1=ln1self, op=ALU.add)
            exp_lrelu(e1s, [H], "h")

            den1 = pers.tile([128, H], f32)
            nc.vector.tensor_reduce(out=den1, in_=e1n, axis=AX.X, op=ALU.add)
            nc.vector.tensor_tensor(out=den1, in0=den1, in1=e1s, op=ALU.add)
            r1 = pers.tile([128, H], f32)
            nc.vector.reciprocal(r1, den1)
            e1nf = pers.tile([128, H, P1], f16)
            nc.vector.tensor_tensor(
                out=e1nf, in0=e1n,
                in1=r1[:, :, None].broadcast_to([128, H, P1]), op=ALU.mult)
            e1sf = pers.tile([128, H], f16)
            nc.vector.tensor_tensor(out=e1sf, in0=e1s, in1=r1, op=ALU.mult)

            z1 = zpool.tile([128, H, F1], f16, tag="z1")
            nc.vector.tensor_tensor(
                out=z1, in0=h0_s[:, None, :].broadcast_to([128, H, F1]),
                in1=e1sf[:, :, None].broadcast_to([128, H, F1]), op=ALU.mult)
            for h in range(H):
                wt1 = tmp512.tile([128, P1, F1], f16, tag=f"t{P1}x{F1}")
                nc.vector.tensor_tensor(
                    out=wt1, in0=h1_s,
                    in1=e1nf[:, h, :][:, :, None].broadcast_to([128, P1, F1]),
                    op=ALU.mult)
                zh1 = zhpool.tile([128, F1], f32, tag="zh1")
                nc.vector.tensor_reduce(out=zh1, in_=wt1.transpose([0, 2, 1]),
                                        axis=AX.X, op=ALU.add)
                nc.vector.tensor_tensor(out=z1[:, h, :], in0=z1[:, h, :],
                                        in1=zh1, op=ALU.add)

            # project with w1 (keep transposed), then final fc, then transpose
            z1T = []
            for h in range(H):
                row = []
                for fc in range(4):
                    pt = psT.tile([128, 128], f16, tag="psT")
                    nc.tensor.transpose(pt, z1[:, h, fc * 128:(fc + 1) * 128],
                                        ident16)
                    zt = ztpool.tile([128, 128], f16, tag="zt")
                    nc.scalar.copy(zt, pt)
                    row.append(zt)
                z1T.append(row)
            hpT = []
            for h in range(H):
                hp = psM.tile([128, 128], f32, tag="psM")
                for fc in range(4):
                    nc.tensor.matmul(hp, w1_t[:, h, fc, :], z1T[h][fc],
                                     start=(fc == 0), stop=(fc == 3))
                ht = htpool.tile([128, 128], f16, tag="ht")
                nc.scalar.copy(ht, hp)
                hpT.append(ht)

            out_s = opool.tile([128, F], f32)
            for mh in range(2):
                po = psM.tile([128, 128], f32, tag="psM")
                for fc in range(4):
                    nc.tensor.matmul(po, fcw_t[:, fc, mh, :], hpT[fc],
                                     start=(fc == 0), stop=(fc == 3))
                ot = htpool.tile([128, 128], f32, tag="ot")
                nc.scalar.copy(ot, po)
                pb = psT.tile([128, 128], f32, tag="psTf32")
                nc.tensor.transpose(pb, ot, ident32)
                nc.scalar.copy(out_s[:, mh * 128:(mh + 1) * 128], pb)

            nc.sync.dma_start(out=out, in_=out_s)

    nc.compile()
    return nc


# ---------------------------------------------------------------------------
# Cached PJRT runner (the axon execution path of run_bass_kernel_spmd,
# with the jit executable built once instead of per call)
# ---------------------------------------------------------------------------

IN_ORDER = ["x2q", "x2s", "x1", "x0", "w0", "a0s", "a0n", "w1", "a1s", "a1n",
            "fcw"]

_runner = None


def _build_runner():
    import jax
    from jax.sharding import Mesh, PartitionSpec
    from jax.experimental.shard_map import shard_map
    from concourse import bass2jax, mybir as mb

    nc = build_program()
    bass2jax.install_neuronx_cc_hook()

    partition_name = (nc.partition_id_tensor.name
                      if nc.partition_id_tensor else None)
    in_names, out_names, out_avals, zero_outs = [], [], [], []
    for alloc in nc.m.functions[0].allocations:
        if not isinstance(alloc, mb.MemoryLocationSet):
            continue
        name = alloc.memorylocations[0].name
        if alloc.kind == "ExternalInput":
            if name != partition_name:
                in_names.append(name)
        elif alloc.kind == "ExternalOutput":
            out_names.append(name)
            shape = tuple(alloc.tensor_shape)
            dtype = mb.dt.np(alloc.dtype)
            out_avals.append(jax.core.ShapedArray(shape, dtype))
            zero_outs.append(np.zeros(shape, dtype))
    n_params = len(in_names)
    n_outs = len(out_avals)
    all_names = list(in_names) + list(out_names)
    if partition_name is not None:
        all_names.append(partition_name)

    def _body(*args):
        operands = list(args)
        if partition_name is not None:
            operands.append(bass2jax.partition_id_tensor())
        outs = bass2jax._bass_exec_p.bind(
            *operands,
            out_avals=tuple(out_avals),
            in_names=tuple(all_names),
            out_names=tuple(out_names),
            lowering_input_output_aliases=(),
            sim_require_finite=True,
            sim_require_nnan=True,
            nc=nc,
        )
        return tuple(outs)

    donate = tuple(range(n_params, n_params + n_outs))
    devices = jax.devices()[:N_CORES]
    mesh = Mesh(np.asarray(devices), ("core",))
    in_specs = (PartitionSpec("core"),) * (n_params + n_outs)
    out_specs = (PartitionSpec("core"),) * n_outs
    sharded = jax.jit(
        shard_map(_body, mesh=mesh, in_specs=in_specs, out_specs=out_specs,
                  check_rep=False),
        donate_argnums=donate, keep_unused=True)

    assert in_names == IN_ORDER, f"input order changed: {in_names}"

    def run(concat_inputs):
        concat_zeros = [np.zeros((N_CORES * z.shape[0], *z.shape[1:]), z.dtype)
                        for z in zero_outs]
        outs = sharded(*concat_inputs, *concat_zeros)
        return np.asarray(outs[0])

    return run


# ---------------------------------------------------------------------------
# Host-side quantization
# ---------------------------------------------------------------------------

def _quantize_x2(x2):
    """Per-row symmetric int8: returns q (B,250,256) int8, scales (B,250) f16."""
    x2 = np.ascontiguousarray(x2.reshape(B, P1 * S2, F), dtype=np.float32)
    q = np.empty(x2.shape, np.int8)
    s = np.empty((B, P1 * S2), np.float32)

    def work(lo, hi):
        blk = x2[lo:hi]
        am = np.abs(blk).max(axis=-1)
        np.maximum(am, 1e-12, out=am)
        sc = am / 127.0
        s[lo:hi] = sc
        t = blk * (1.0 / sc)[..., None]
        np.rint(t, out=t)
        q[lo:hi] = t.astype(np.int8)

    n_thr = 8
    step = B // n_thr
    threads = [threading.Thread(target=work, args=(i * step, (i + 1) * step))
               for i in range(n_thr)]
    for t in threads:
        t.start()
    for t in threads:
        t.join()
    return q, s.astype(np.float16)


def _rep(a):
    """Replicate a param for all cores along axis 0 (shard_map splits axis 0)."""
    a = np.asarray(a, np.float16)
    return np.ascontiguousarray(np.broadcast_to(a[None], (N_CORES,) + a.shape)
                                .reshape((N_CORES * a.shape[0],) + a.shape[1:]))


def kernel(x0, x1, x2, w0_fc, a0_self, a0_neigh, w1_fc, a1_self, a1_neigh,
           fc_w):
    global _runner
    if _runner is None:
        _runner = _build_runner()

    q, s16 = _quantize_x2(np.asarray(x2))
    inputs = [
        q,
        s16,
        np.asarray(x1, np.float16).reshape(B, P1, F),
        np.asarray(x0, np.float16).reshape(B, F),
        _rep(w0_fc), _rep(a0_self), _rep(a0_neigh),
        _rep(w1_fc), _rep(a1_self), _rep(a1_neigh),
        _rep(fc_w),
    ]
    out = _runner(inputs)
    return np.ascontiguousarray(out.astype(np.float32))
